# revision 18
# baseline (speedup 1.0000x reference)
"""DA-RNN Trainium2 Bass kernel (v2: 2-way batch-group interleave).

Data-parallel over batch: 256 batch / 8 cores = 32 per core.  The per-core
batch is split into two groups of 16 that run the recurrence interleaved, so
one group's cross-engine semaphore gaps are filled by the other group's work.

Temporal attention is deferred out of the step chain: h_t (bf16) is written
into a 32-slot ring `hist`; every 16 steps one block is processed with a
batched ts matmul ([1, 512]), tanh/exp halves on ACT, a PE broadcast, the
e*h multiply on the (otherwise idle) Pool engine and a tau-reduction on DVE.
Block work is scattered one op per step into engine slack.

The next step's score matmuls read qn and p2 directly (W_a distributed over
h = qn + 0.5*p2) so the chain restarts before hm_new lands.  Gate math reads
PSUM directly (no evictions), sigmoid(x) = 0.5 + 0.5*tanh(x/2) keeps all
activations inside the exp_and_others table set.
"""

from collections import defaultdict

import numpy as np

import concourse.bass as bass
import concourse.mybir as mybir
import concourse.tile as tile
from concourse import bacc
from concourse.bass_utils import run_bass_kernel_spmd

F32 = mybir.dt.float32
BF16 = mybir.dt.bfloat16
AF = mybir.ActivationFunctionType
ALU = mybir.AluOpType
AX = mybir.AxisListType

B, S, I, H, O = 256, 512, 128, 256, 1
NCORES = 8
BL = B // NCORES   # 32 local batch
GB = BL // 2       # 16 per group
RING = 32          # hist ring slots
POOL_ENGINE = True  # False routes Pool ops to DVE (debug)
BLK = 16           # temporal-attention block size


def _build_program(n_steps: int):
    nc = bacc.Bacc(None, target_bir_lowering=False)

    # ---- DRAM I/O (per-core shapes; weights replicated across cores) ----
    x_d = nc.dram_tensor("x", [128, n_steps * BL], F32, kind="ExternalInput")
    wax_d = nc.dram_tensor("wax", [128, 128], F32, kind="ExternalInput")
    wah_d = nc.dram_tensor("wah", [128, 256], F32, kind="ExternalInput")
    wih_d = nc.dram_tensor("wih", [128, 768], F32, kind="ExternalInput")
    whh_d = nc.dram_tensor("whh", [128, 1536], F32, kind="ExternalInput")
    b8_d = nc.dram_tensor("b8", [8, 128], F32, kind="ExternalInput")
    sel8_d = nc.dram_tensor("sel8", [8, 128], F32, kind="ExternalInput")
    ba_d = nc.dram_tensor("ba", [128, 1], F32, kind="ExternalInput")
    wt_d = nc.dram_tensor("wt", [128, 2], F32, kind="ExternalInput")
    bt_d = nc.dram_tensor("bt", [1, 1], F32, kind="ExternalInput")
    wf_d = nc.dram_tensor("wf", [128, 2], F32, kind="ExternalInput")
    bf_d = nc.dram_tensor("bf", [1, 1], F32, kind="ExternalInput")  # 0.5*b_f
    out_d = nc.dram_tensor("out", [1, BL], F32, kind="ExternalOutput")

    n_blocks = (n_steps + BLK - 1) // BLK

    with tile.TileContext(nc) as tc:
        with (
            tc.tile_pool(name="big", bufs=1) as big,
            tc.tile_pool(name="wpool", bufs=1) as wpool,
            tc.tile_pool(name="state", bufs=1) as state,
            tc.tile_pool(name="work", bufs=2) as work,
            tc.tile_pool(name="ps_sc", bufs=1, space="PSUM") as p_sc_pool,
            tc.tile_pool(name="ps_g0", bufs=1, space="PSUM") as p_g0_pool,
            tc.tile_pool(name="ps_g1", bufs=1, space="PSUM") as p_g1_pool,
            tc.tile_pool(name="ps_A", bufs=1, space="PSUM") as p_A_pool,
            tc.tile_pool(name="ps_np", bufs=1, space="PSUM") as p_np_pool,
            tc.tile_pool(name="ps_ts", bufs=1, space="PSUM") as p_ts_pool,
            tc.tile_pool(name="ps_ebc", bufs=1, space="PSUM") as p_ebc_pool,
        ):
            # ACT table warmup (first tanh/exp trigger the table load).
            warm = state.tile([1, 2], F32, tag="warm")
            nc.gpsimd.memset(warm[:], 0.0)
            nc.scalar.activation(warm[:], warm[:], AF.Tanh)
            nc.scalar.activation(warm[:], warm[:], AF.Exp)

            # ---- load inputs ----
            x_sb = big.tile([128, n_steps * BL], F32)
            nchunk = 8 if (n_steps * BL) % 8 == 0 else 1
            cw = (n_steps * BL) // nchunk
            for c in range(nchunk):
                nc.sync.dma_start(x_sb[:, c * cw:(c + 1) * cw],
                                  x_d[:, c * cw:(c + 1) * cw])

            def load_w(dram, shape, name):
                t = wpool.tile(shape, F32, tag=name)
                nc.sync.dma_start(t[:], dram[:])
                return t

            wax = load_w(wax_d, [128, 128], "wax")       # stays f32
            wah = load_w(wah_d, [128, 256], "wah")
            wih = load_w(wih_d, [128, 768], "wih")
            whh = load_w(whh_d, [128, 1536], "whh")
            b8 = load_w(b8_d, [8, 128], "b8")
            sel8 = load_w(sel8_d, [8, 128], "sel8")
            ba = load_w(ba_d, [128, 1], "ba")
            wt = load_w(wt_d, [128, 2], "wt")
            bt = load_w(bt_d, [1, 1], "bt")
            wf = load_w(wf_d, [128, 2], "wf")
            bf = load_w(bf_d, [1, 1], "bf")

            def to_bf(t, shape, name):
                tm = wpool.tile(shape, BF16, tag=name)
                nc.vector.tensor_copy(tm[:], t[:])
                return tm

            wah_m = to_bf(wah, [128, 256], "wah_m")
            wap_m = wpool.tile([128, 256], BF16, tag="wap_m")  # 0.5*wah
            nc.vector.tensor_scalar(wap_m[:], wah[:], 0.5, None, ALU.mult)
            wih_m = to_bf(wih, [128, 768], "wih_m")
            whh_m = to_bf(whh, [128, 1536], "whh_m")
            b8_m = to_bf(b8, [8, 128], "b8_m")
            sel8_m = to_bf(sel8, [8, 128], "sel8_m")
            wt_m = to_bf(wt, [128, 2], "wt_m")
            wf_m = to_bf(wf, [128, 2], "wf_m")

            ones_col = state.tile([128, 1], BF16, tag="ones_col")
            nc.vector.memset(ones_col[:], 1.0)
            ones_row_f = state.tile([1, 128], F32, tag="ones_row_f")
            nc.vector.memset(ones_row_f[:], 1.0)
            ones_row_b = state.tile([1, 128], BF16, tag="ones_row_b")
            nc.vector.memset(ones_row_b[:], 1.0)
            halfs = state.tile([128, 2, GB], BF16, tag="halfs")
            nc.vector.memset(halfs[:], 0.5)

            # ---- persistent state ----
            hist = big.tile([128, 2, BL, RING], BF16, tag="hist")
            # layout (k, b, slot): h_t slice = hist[:, :, g*GB:+GB, slot]
            U = state.tile([128, 2, BL], F32, tag="U")
            nc.vector.memset(U[:], 0.0)
            Z = state.tile([1, BL], F32, tag="Z")
            nc.vector.memset(Z[:], 0.0)
            Ublk = state.tile([128, 2, BL], F32, tag="Ublk")
            Zblk = state.tile([1, BL], F32, tag="Zblk")
            ts_sb = state.tile([1, BL * BLK], F32, tag="ts_sb")
            e_all = state.tile([1, BL * BLK], BF16, tag="e_all")
            ebc_sb = state.tile([128, BL * BLK], BF16, tag="ebc_sb")
            tmp4 = big.tile([128, 2, BL, BLK], BF16, tag="tmp4")

            # ---- persistent PSUM tiles (8 banks) ----
            ps_sc = p_sc_pool.tile([128, 2 * GB], F32)
            ps_g0 = p_g0_pool.tile([128, 8, GB], F32)
            ps_g1 = p_g1_pool.tile([128, 8, GB], F32)
            ps_g = [ps_g0, ps_g1]
            psA = p_A_pool.tile([128, 64], F32)   # bc [:,0:32], sum [0:1,32:64]
            ps_np = p_np_pool.tile([128, 4, GB], F32)
            ps_ts = p_ts_pool.tile([1, BL * BLK], F32)
            ps_ebc = p_ebc_pool.tile([128, BL * BLK], F32)

            MM = nc.tensor.matmul
            pool = nc.gpsimd if POOL_ENGINE else nc.vector

            # ---- deferred temporal-attention block ops -------------------
            sched = defaultdict(lambda: defaultdict(list))
            tail_ops = []

            def emit_block(j):
                """Deferred ops for block j; flat packing col = b*r + tau."""
                base = (j % 2) * BLK
                t0 = j * BLK
                r = min(BLK, n_steps - t0)
                ops = []

                def ts_mm():
                    for k in range(2):
                        MM(ps_ts[0:1, 0:BL * r], wt_m[:, k:k + 1],
                           hist[:, k, :, base:base + r],
                           start=(k == 0), stop=(k == 1))
                ops.append((0, "PE", ts_mm))

                def th_half(bh):
                    def f():
                        c0, c1 = bh * GB * r, (bh + 1) * GB * r
                        nc.scalar.activation(ts_sb[0:1, c0:c1],
                                             ps_ts[0:1, c0:c1],
                                             AF.Tanh, bias=bt[:, 0:1])
                    return f
                ops.append((1, "ACT", th_half(0)))
                ops.append((2, "ACT", th_half(1)))

                def ex_half(bh):
                    def f():
                        c0, c1 = bh * GB * r, (bh + 1) * GB * r
                        nc.scalar.activation(e_all[0:1, c0:c1],
                                             ts_sb[0:1, c0:c1], AF.Exp)
                    return f
                ops.append((3, "ACT", ex_half(0)))
                ops.append((4, "ACT", ex_half(1)))

                def ebc_mm():
                    MM(ps_ebc[:, 0:BL * r], ones_row_b[:],
                       e_all[0:1, 0:BL * r], start=True, stop=True)
                ops.append((5, "PE", ebc_mm))

                def ebc_ev(bh):
                    def f():
                        c0, c1 = bh * GB * r, (bh + 1) * GB * r
                        nc.scalar.activation(ebc_sb[:, c0:c1],
                                             ps_ebc[:, c0:c1], AF.Copy)
                    return f
                ops.append((6, "ACT", ebc_ev(0)))
                ops.append((7, "ACT", ebc_ev(1)))

                d = 8
                for k in range(2):
                    for bh in range(2):
                        def mul_q(k=k, bh=bh):
                            c0 = bh * GB * r
                            ebc_v = ebc_sb[:, c0:c0 + GB * r].rearrange(
                                "p (a b) -> p a b", a=GB)
                            pool.tensor_tensor(
                                tmp4[:, k, bh * GB:(bh + 1) * GB, 0:r],
                                hist[:, k, bh * GB:(bh + 1) * GB,
                                     base:base + r],
                                ebc_v, op=ALU.mult)
                        ops.append((d, "POOL", mul_q))

                        if r == BLK:
                            # contiguous inner dim -> hw tensor_reduce ok
                            def red_q(k=k, bh=bh):
                                nc.vector.tensor_reduce(
                                    Ublk[:, k, bh * GB:(bh + 1) * GB],
                                    tmp4[:, k, bh * GB:(bh + 1) * GB, 0:r],
                                    axis=AX.X, op=ALU.add)
                            ops.append((d + 1, "DVE", red_q))
                        d += 1
                if r < BLK:
                    # tail-only partial block: accumulate U directly
                    for tau in range(r):
                        def uadd(tau=tau):
                            nc.vector.tensor_tensor(
                                U[:], U[:], tmp4[:, :, :, tau], op=ALU.add)
                        ops.append((d + 1 + tau, "DVE", uadd))

                def zred():
                    e_v = e_all[0:1, 0:BL * r].rearrange(
                        "p (a b) -> p a b", a=BL)
                    nc.vector.tensor_reduce(Zblk[:], e_v, axis=AX.X,
                                            op=ALU.add)
                ops.append((6, "DVE", zred))

                def zacc():
                    nc.vector.tensor_tensor(Z[:], Z[:], Zblk[:], op=ALU.add)
                ops.append((7, "DVE", zacc))

                if r == BLK:
                    def uacc():
                        nc.vector.tensor_tensor(U[:], U[:], Ublk[:],
                                                op=ALU.add)
                    ops.append((12, "DVE", uacc))

                fire = t0 + r  # first step whose stream can host these ops
                for d, eng, fn in ops:
                    step = fire + d
                    if step < n_steps:
                        sched[step][eng].append(fn)
                    else:
                        tail_ops.append((fire + d, len(tail_ops), eng, fn))

            for j in range(n_blocks):
                emit_block(j)

            # ---- main loop ----
            qn_prev = [None, None]
            p2_prev = [None, None]

            for t in range(n_steps):
                slot = t % RING
                pslot = (t - 1) % RING

                def xsl(g):
                    return x_sb[:, t * BL + g * GB: t * BL + (g + 1) * GB]

                exp_sb = [work.tile([128, GB], BF16, tag=f"exp{g}", name=f"exp{g}")
                          for g in range(2)]
                y_t = [work.tile([128, GB], BF16, tag=f"y{g}", name=f"y{g}")
                       for g in range(2)]
                rsum = [work.tile([1, GB], F32, tag=f"rsum{g}", name=f"rsum{g}")
                        for g in range(2)]
                xw = [work.tile([128, GB], BF16, tag=f"xw{g}", name=f"xw{g}")
                      for g in range(2)]
                srz = [work.tile([128, 4, GB], BF16, tag=f"srz{g}", name=f"srz{g}")
                       for g in range(2)]
                nm = [work.tile([128, 2, GB], BF16, tag=f"nm{g}", name=f"nm{g}")
                      for g in range(2)]
                n_sb = [work.tile([128, 2, GB], BF16, tag=f"n{g}", name=f"n{g}")
                        for g in range(2)]
                q_t = [work.tile([128, 2, GB], BF16, tag=f"q{g}", name=f"q{g}")
                       for g in range(2)]
                p2_t = [work.tile([128, 2, GB], BF16, tag=f"p2{g}", name=f"p2{g}")
                        for g in range(2)]
                qn_t = [work.tile([128, 2, GB], BF16, tag=f"qn{g}", name=f"qn{g}")
                        for g in range(2)]

                # Emission order respects dataflow (tile deps follow program
                # order); per-engine projections give the intended queue
                # orders that let the two groups interleave.

                # -- PE: scores (x + prev-step qn/p2), biases, gh
                for g in range(2):
                    sc = ps_sc[:, g * GB:(g + 1) * GB]
                    MM(sc, wax[:], xsl(g), start=True, stop=(t == 0))
                    if t >= 1:
                        for k in range(2):
                            MM(sc, wah_m[:, k * 128:(k + 1) * 128],
                               qn_prev[g][:, k, :],
                               start=False, stop=(t == 1 and k == 1))
                    if t >= 2:
                        for k in range(2):
                            MM(sc, wap_m[:, k * 128:(k + 1) * 128],
                               p2_prev[g][:, k, :],
                               start=False, stop=(k == 1))
                # bank blocks: 0-3 rz, 4-5 i_n, 6-7 h_n; one open
                # accumulation group per bank (closed by the last gi MM)
                for g in range(2):
                    MM(ps_g[g][:, :, :], b8_m[:], sel8_m[:],
                       start=True, stop=False)
                    if t >= 1:
                        for m in range(4):
                            for k in range(2):
                                MM(ps_g[g][:, m, :],
                                   whh_m[:, k * 768 + m * 128:
                                         k * 768 + (m + 1) * 128],
                                   hist[:, k, g * GB:(g + 1) * GB, pslot],
                                   start=False, stop=False)
                        for m in range(2):
                            for k in range(2):
                                MM(ps_g[g][:, 6 + m, :],
                                   whh_m[:, k * 768 + (4 + m) * 128:
                                         k * 768 + (5 + m) * 128],
                                   hist[:, k, g * GB:(g + 1) * GB, pslot],
                                   start=False, stop=False)

                # -- ACT: tanh(scores) then exp
                for g in range(2):
                    sc = ps_sc[:, g * GB:(g + 1) * GB]
                    nc.scalar.activation(sc, sc, AF.Tanh, bias=ba[:, 0:1])
                for g in range(2):
                    nc.scalar.activation(exp_sb[g][:],
                                         ps_sc[:, g * GB:(g + 1) * GB],
                                         AF.Exp)

                # -- POOL: y = e * x
                for g in range(2):
                    pool.tensor_tensor(y_t[g][:], exp_sb[g][:], xsl(g),
                                            op=ALU.mult)

                # -- PE sum, DVE recip, PE bc, DVE xw, PE gi
                for g in range(2):
                    MM(psA[0:1, 32 + g * GB:32 + (g + 1) * GB], ones_col[:],
                       exp_sb[g][:], start=True, stop=True)
                for g in range(2):
                    nc.vector.reciprocal(
                        rsum[g][:], psA[0:1, 32 + g * GB:32 + (g + 1) * GB])
                for g in range(2):
                    MM(psA[:, g * GB:(g + 1) * GB], ones_row_f[:],
                       rsum[g][:], start=True, stop=True)
                for g in range(2):
                    nc.vector.tensor_tensor(xw[g][:], y_t[g][:],
                                            psA[:, g * GB:(g + 1) * GB],
                                            op=ALU.mult)
                for g in range(2):
                    for m in range(4):
                        MM(ps_g[g][:, m, :],
                           wih_m[:, m * 128:(m + 1) * 128], xw[g][:],
                           start=False, stop=False)
                    for m in range(2):
                        MM(ps_g[g][:, 4 + m, :],
                           wih_m[:, (4 + m) * 128:(5 + m) * 128], xw[g][:],
                           start=False, stop=(m == 1))

                # -- ACT: gates r,z
                for g in range(2):
                    nc.scalar.activation(srz[g][:],
                                         ps_g[g][:, 0:4, :],
                                         AF.Tanh, scale=0.5)

                # -- DVE: n-path; POOL: z-path precomputes
                for g in range(2):
                    nc.vector.scalar_tensor_tensor(
                        nm[g][:], srz[g][:, 0:2, :], 1.0,
                        ps_g[g][:, 6:8, :],
                        op0=ALU.add, op1=ALU.mult)
                    nc.vector.tensor_tensor(ps_np[:, g * 2:(g + 1) * 2, :],
                                            nm[g][:],
                                            ps_g[g][:, 4:6, :],
                                            op=ALU.add)
                for g in range(2):
                    ht = work.tile([128, 2, GB], BF16, tag=f"ht{g}",
                                   name=f"ht{g}")
                    pool.tensor_tensor(ht[:], halfs[:], srz[g][:, 2:4, :],
                                       op=ALU.mult)
                    pool.tensor_tensor(q_t[g][:], halfs[:], ht[:],
                                       op=ALU.subtract)
                    if t >= 1:
                        hprev = hist[:, :, g * GB:(g + 1) * GB, pslot]
                        pa = work.tile([128, 2, GB], BF16, tag=f"pa{g}",
                                       name=f"pa{g}")
                        pool.tensor_tensor(pa[:], srz[g][:, 2:4, :],
                                           hprev, op=ALU.mult)
                        pool.tensor_tensor(p2_t[g][:], pa[:], hprev,
                                           op=ALU.add)

                # -- ACT: tanh(n); DVE: qn, h_new
                for g in range(2):
                    nc.scalar.activation(n_sb[g][:],
                                         ps_np[:, g * 2:(g + 1) * 2, :],
                                         AF.Tanh)
                for g in range(2):
                    nc.vector.tensor_tensor(qn_t[g][:], q_t[g][:],
                                            n_sb[g][:], op=ALU.mult)
                for g in range(2):
                    hm_out = hist[:, :, g * GB:(g + 1) * GB, slot]
                    if t == 0:
                        nc.vector.tensor_copy(hm_out, qn_t[g][:])
                    else:
                        nc.vector.scalar_tensor_tensor(
                            hm_out, p2_t[g][:], 0.5, qn_t[g][:],
                            op0=ALU.mult, op1=ALU.add)

                # -- deferred temporal-attention block ops in this step's slack
                for eng in ("PE", "ACT", "DVE", "POOL"):
                    for fn in sched[t][eng]:
                        fn()

                qn_prev = qn_t
                p2_prev = p2_t

            # ---- tail: remaining block ops in intended order ----
            for _, _, eng, fn in sorted(tail_ops, key=lambda x: (x[0], x[1])):
                fn()

            # ---- context = U / Z ; out = sigmoid(W_f @ ctx + b_f) ----
            rZ = state.tile([1, BL], F32, tag="rZ")
            nc.vector.reciprocal(rZ[:], Z[:])
            MM(psA[:, 0:32], ones_row_f[:], rZ[:], start=True, stop=True)
            ctx = state.tile([128, 2, BL], BF16, tag="ctx")
            for k in range(2):
                nc.vector.tensor_tensor(ctx[:, k, :], U[:, k, :],
                                        psA[:, 0:32], op=ALU.mult)
            for k in range(2):
                MM(psA[0:1, 32:64], wf_m[:, k:k + 1], ctx[:, k, :],
                   start=(k == 0), stop=(k == 1))
            sig_t = state.tile([1, BL], F32, tag="sig_t")
            nc.scalar.activation(sig_t[:], psA[0:1, 32:64], AF.Tanh,
                                 bias=bf[:, 0:1], scale=0.5)
            out_sb = state.tile([1, BL], F32, tag="out_sb")
            nc.vector.tensor_scalar(out_sb[:], sig_t[:], 0.5, 0.5,
                                    ALU.mult, ALU.add)
            nc.sync.dma_start(out_d[:], out_sb[:])

    return nc


_PROGRAM_CACHE = {}


def _get_program(n_steps: int):
    if n_steps not in _PROGRAM_CACHE:
        nc = _build_program(n_steps)
        nc.finalize()
        _PROGRAM_CACHE[n_steps] = nc
    return _PROGRAM_CACHE[n_steps]


def _prep_weights(W_a, b_a, W_ih, b_ih, W_hh, b_hh, W_t, b_t, W_f, b_f):
    f = np.float32
    wax = np.ascontiguousarray(W_a[:, 0:128].T).astype(f)
    wah = np.concatenate([W_a[:, 128:256].T, W_a[:, 256:384].T],
                         axis=1).astype(f)
    wih = np.ascontiguousarray(W_ih.T).astype(f)
    W_hh_s = np.array(W_hh, dtype=f)
    W_hh_s[512:768, :] *= 0.5    # pre-halve n-gate h-side
    whh = np.concatenate([W_hh_s[:, 0:128].T, W_hh_s[:, 128:256].T],
                         axis=1).astype(f)
    b8 = np.concatenate([
        np.asarray(b_ih + b_hh)[0:512].reshape(4, 128),
        np.asarray(b_ih)[512:768].reshape(2, 128),
        (0.5 * np.asarray(b_hh)[512:768]).reshape(2, 128),
    ], axis=0).astype(f)
    sel8 = np.zeros((8, 128), f)
    for m in range(8):
        sel8[m, m * GB:(m + 1) * GB] = 1.0
    ba = np.asarray(b_a).reshape(128, 1).astype(f)
    wt = np.asarray(W_t).reshape(2, 128).T.astype(f)
    bt = np.array([[float(np.asarray(b_t).reshape(()))]], dtype=f)
    wf = np.asarray(W_f).reshape(2, 128).T.astype(f)
    bf = np.array([[0.5 * float(np.asarray(b_f).reshape(()))]], dtype=f)
    return dict(wax=wax, wah=wah, wih=wih, whh=whh, b8=b8, sel8=sel8,
                ba=ba, wt=wt, bt=bt, wf=wf, bf=bf)


def kernel(x, W_a, b_a, W_ih, b_ih, W_hh, b_hh, W_t, b_t, W_f, b_f,
           n_steps=None, trace=False):
    x = np.asarray(x, dtype=np.float32)
    n_steps = x.shape[1] if n_steps is None else n_steps
    nc = _get_program(n_steps)
    wmap = _prep_weights(np.asarray(W_a), np.asarray(b_a), np.asarray(W_ih),
                         np.asarray(b_ih), np.asarray(W_hh), np.asarray(b_hh),
                         np.asarray(W_t), np.asarray(b_t), np.asarray(W_f),
                         np.asarray(b_f))
    in_maps = []
    nb = x.shape[0] // NCORES
    for c in range(NCORES):
        xc = x[c * nb:(c + 1) * nb]                              # [nb, S, I]
        xf = np.ascontiguousarray(xc.transpose(2, 1, 0)).reshape(128, -1)
        m = dict(wmap)
        m["x"] = np.ascontiguousarray(xf, dtype=np.float32)
        in_maps.append(m)
    res = run_bass_kernel_spmd(nc, in_maps, core_ids=list(range(NCORES)),
                               trace=trace)
    out = np.zeros((x.shape[0], 1), dtype=np.float32)
    for c in range(NCORES):
        out[c * nb:(c + 1) * nb, 0] = res.results[c]["out"][0, :]
    kernel.last_results = res
    return out
